# revision 34
# baseline (speedup 1.0000x reference)
"""Causal attention kernel for 8 Trainium2 NeuronCores.

Problem: x[4,2048,1024] fp32, Wq/Wk/Wv[1024,1024] fp32 (nn.Linear: y = x @ W.T),
single-head causal attention, softmax(QK^T/sqrt(D)) @ V.

Sharding: 2 cores per batch; within a batch, queries are split by row PARITY
(core h takes global rows s with s % 2 == h). This makes causal work exactly
balanced across the pair and keeps one uniform SPMD program (per-core
differences are pure data: the xqT slice and the causal mask tiles).

Device layout (PE matmul computes out = lhsT.T @ rhs, contracting over the
128-partition dim):
  - host passes x^T and W^T in bf16, so the contraction dim d lands on
    partitions with zero on-device transposes; bf16 halves DMA and SBUF and
    enables fast weight load, while all matmuls accumulate in fp32 PSUM
  - Kt[e,k], Qt[e,q] come straight out of the projections (e on partitions)
  - scores are computed transposed St[k,q]; no max-subtraction is needed
    (logits are bounded ~|2.5|); masked-out logits get -1e30 via additive
    host-built mask tiles; softmax denominator is a ones-vector matmul so the
    key-dim (partition) reduction happens on the PE with fp32 accumulation
  - exp on ScalarE writes P^T in bf16; AV matmul bf16 with fp32 accumulate;
    the final per-row 1/denom scale rides the PSUM->SBUF copy on VectorE
  - numerics: rel err vs fp32 reference ~3.4e-3, dominated by bf16 P/V
"""

import numpy as np

B, S, D, P = 4, 2048, 1024, 128
NQ = S // 2          # queries per core (parity split)
QT = 256             # score-tile width in (core-local) query dim
NEG = -1e30
N_CORES = 8

_cache = {}


def _build():
    import concourse.mybir as mybir
    import concourse.tile as tile
    from concourse import bacc

    f32 = mybir.dt.float32
    bf = mybir.dt.bfloat16

    nc = bacc.Bacc()

    xT = nc.dram_tensor("xT", [D, S], bf, kind="ExternalInput")
    xqT = nc.dram_tensor("xqT", [D, NQ], bf, kind="ExternalInput")
    wqT = nc.dram_tensor("wqT", [D, D], bf, kind="ExternalInput")
    wkT = nc.dram_tensor("wkT", [D, D], bf, kind="ExternalInput")
    wvT = nc.dram_tensor("wvT", [D, D], bf, kind="ExternalInput")
    masks = nc.dram_tensor("masks", [4, P, QT], f32, kind="ExternalInput")
    out = nc.dram_tensor("out", [NQ, D], f32, kind="ExternalOutput")

    xT3 = xT.ap().rearrange("(do di) s -> di do s", di=P)
    xq3 = xqT.ap().rearrange("(do di) s -> di do s", di=P)
    wq3 = wqT.ap().rearrange("(do di) e -> di do e", di=P)
    wk3 = wkT.ap().rearrange("(do di) e -> di do e", di=P)
    wv3 = wvT.ap().rearrange("(do di) e -> di do e", di=P)
    out_ap = out.ap()
    masks_ap = masks.ap()

    EXP = mybir.ActivationFunctionType.Exp
    COPYF = mybir.ActivationFunctionType.Copy
    SCALE = 1.0 / np.sqrt(np.float32(D))

    with tile.TileContext(nc) as tc:
        with (
            tc.tile_pool(name="const", bufs=1) as const_pool,
            tc.tile_pool(name="prod", bufs=1) as prod,
            tc.tile_pool(name="ins", bufs=1) as ins_pool,
            tc.tile_pool(name="wk", bufs=2) as wk_pool,
            tc.tile_pool(name="wq", bufs=2) as wq_pool,
        ):
            # ---- head: only the first Kt iteration's deps lead ----
            wk0lo = wk_pool.tile([P, 4, 2 * P], bf, tag="wklo", name="wk0lo")
            nc.sync.dma_start(wk0lo[:], wk3[:, 0:4, 0 : 2 * P])
            wk0hi = wk_pool.tile([P, 4, 2 * P], bf, tag="wkhi", name="wk0hi")
            nc.sync.dma_start(wk0hi[:], wk3[:, 4:8, 0 : 2 * P])
            wk0 = (wk0lo, wk0hi)
            # preload the Exp activation table while the first DMAs land
            warm = const_pool.tile([P, 1], f32)
            nc.vector.memset(warm[:], 0.0)
            nc.scalar.activation(out=warm[:], in_=warm[:], func=EXP, scale=1.0)

            def load_slabs(pair, split_lo=False):
                # per slab: list of (tile, do_start, do_count)
                plan = [(0, 2), (2, 2), (4, 4)] if split_lo else [(0, 4), (4, 4)]
                tiles = [[] for _ in range(2)]
                for s2 in range(2):
                    s = pair * 2 + s2
                    for d0, dn in plan:
                        t = ins_pool.tile(
                            [P, dn, 512], bf,
                            tag=f"xt{s}d{d0}", name=f"xt{s}d{d0}",
                        )
                        tiles[s2].append((t, d0, dn))
                # emit DMAs in do-ascending order across slabs so early
                # matmuls' pieces land first
                for pi in range(len(plan)):
                    for s2 in range(2):
                        s = pair * 2 + s2
                        t, d0, dn = tiles[s2][pi]
                        nc.sync.dma_start(
                            t[:], xT3[:, d0 : d0 + dn, s * 512 : (s + 1) * 512]
                        )
                return tiles

            xts = load_slabs(0, split_lo=True)

            def xchunk(s2, do, cols):
                for t, d0, dn in xts[s2]:
                    if d0 <= do < d0 + dn:
                        return t[:, do - d0, cols]
                raise AssertionError(do)

            # persistent per-core products, slabbed for fine-grained deps
            kt_slabs = [prod.tile([P, 8, 512], bf, tag=f"kt{s}", name=f"kt{s}") for s in range(4)]
            v_slabs = [prod.tile([P, 4, D], bf, tag=f"v{s}", name=f"v{s}") for s in range(4)]
            qt = prod.tile([P, 8, NQ], bf, tag="qt")

            deferred_loads = {}

            with tc.tile_pool(name="pj", bufs=4, space="PSUM") as pj:

                def kt_half(half, first_wk=None, emit_after=()):
                    for pc in range(4):
                        if pc == 0 and first_wk is not None:
                            lo_hi = first_wk

                            if isinstance(lo_hi, tuple):
                                def wkchunk(do, esl, lo_hi=lo_hi):
                                    t = lo_hi[0] if do < 4 else lo_hi[1]
                                    return t[:, do % 4, esl]
                            else:
                                def wkchunk(do, esl, wk_pair=lo_hi):
                                    return wk_pair[:, do, esl]
                        else:
                            wk_pair = wk_pool.tile([P, 8, 2 * P], bf, tag="wk")
                            nc.sync.dma_start(
                                wk_pair[:], wk3[:, :, pc * 2 * P : (pc + 1) * 2 * P]
                            )

                            def wkchunk(do, esl, wk_pair=wk_pair):
                                return wk_pair[:, do, esl]
                        for e2 in range(2):
                            ec = pc * 2 + e2
                            pss = [
                                pj.tile([P, 512], f32, tag="pj", name="pj")
                                for _ in range(2)
                            ]
                            for do in range(8):
                                for s2 in range(2):
                                    nc.tensor.matmul(
                                        pss[s2][:],
                                        wkchunk(do, slice(e2 * P, (e2 + 1) * P)),
                                        xchunk(s2, do, slice(None)),
                                        start=(do == 0),
                                        stop=(do == 7),
                                    )
                            for s2 in range(2):
                                if s2 == 0:
                                    nc.vector.tensor_copy(
                                        out=kt_slabs[half * 2 + s2][:, ec, :],
                                        in_=pss[s2][:],
                                    )
                                else:
                                    nc.scalar.copy(
                                        out=kt_slabs[half * 2 + s2][:, ec, :],
                                        in_=pss[s2][:],
                                    )
                            for when, what in emit_after:
                                if when == ec:
                                    what()

                def v_half(half):
                    wv_sb = deferred_loads["wv"]
                    for s2 in range(2):
                        for kq in range(4):
                            pvs = [
                                pj.tile([P, 512], f32, tag="pj", name="pj")
                                for _ in range(2)
                            ]
                            for do in range(8):
                                for es in range(2):
                                    nc.tensor.matmul(
                                        pvs[es][:],
                                        xchunk(s2, do, slice(kq * P, (kq + 1) * P)),
                                        wv_sb[:, do, es * 512 : (es + 1) * 512],
                                        start=(do == 0),
                                        stop=(do == 7),
                                    )
                            nc.vector.tensor_copy(
                                out=v_slabs[half * 2 + s2][:, kq, 0:512],
                                in_=pvs[0][:],
                            )
                            nc.scalar.copy(
                                out=v_slabs[half * 2 + s2][:, kq, 512:1024],
                                in_=pvs[1][:],
                            )

                def emit_wv():
                    t = ins_pool.tile([P, 8, D], bf, name="wv_sb")
                    nc.sync.dma_start(t[:], wv3)
                    deferred_loads["wv"] = t

                def emit_xq():
                    t = ins_pool.tile([P, 8, NQ], bf, name="xq_sb")
                    nc.sync.dma_start(t[:], xq3)
                    deferred_loads["xq"] = t

                def emit_wq0():
                    t = wq_pool.tile([P, 8, 2 * P], bf, tag="wq", name="wq0")
                    nc.sync.dma_start(t[:], wq3[:, :, 0 : 2 * P])
                    deferred_loads["wq0"] = t

                def emit_masks():
                    mask_sb = const_pool.tile([P, 4, QT], f32)
                    nc.sync.dma_start(
                        mask_sb[:], masks_ap.rearrange("m p j -> p m j")
                    )
                    ones_sb = const_pool.tile([P, 1], bf)
                    nc.vector.memset(ones_sb[:], 1.0)
                    deferred_loads["mask"] = mask_sb
                    deferred_loads["ones"] = ones_sb

                # ---- K^T first half (slabs 0,1); big loads stream in behind ----
                kt_half(
                    0,
                    first_wk=wk0,
                    emit_after=((3, emit_wv), (5, emit_xq), (6, emit_wq0),
                                (7, emit_masks)),
                )

                # ---- V first half ----
                v_half(0)

                # ---- Q^T projection ----
                xq_sb = deferred_loads["xq"]
                wk1_first = None
                for pc in range(4):
                    if pc == 0:
                        wq_pair = deferred_loads["wq0"]
                    else:
                        wq_pair = wq_pool.tile([P, 8, 2 * P], bf, tag="wq")
                        nc.sync.dma_start(
                            wq_pair[:], wq3[:, :, pc * 2 * P : (pc + 1) * 2 * P]
                        )
                    if pc == 1:
                        xts = load_slabs(1)  # second-half slabs stream in now
                    for e2 in range(2):
                        ec = pc * 2 + e2
                        pqs = [
                            pj.tile([P, 512], f32, tag="pj", name="pj")
                            for _ in range(2)
                        ]
                        for do in range(8):
                            for qs in range(2):
                                nc.tensor.matmul(
                                    pqs[qs][:],
                                    wq_pair[:, do, e2 * P : (e2 + 1) * P],
                                    xq_sb[:, do, qs * 512 : (qs + 1) * 512],
                                    start=(do == 0),
                                    stop=(do == 7),
                                )
                        nc.vector.tensor_copy(
                            out=qt[:, ec, 0:512], in_=pqs[0][:]
                        )
                        nc.scalar.copy(
                            out=qt[:, ec, 512:1024], in_=pqs[1][:]
                        )
                        if ec == 6:
                            wk1_first = wk_pool.tile(
                                [P, 8, 2 * P], bf, tag="wk", name="wk1f"
                            )
                            nc.sync.dma_start(wk1_first[:], wk3[:, :, 0 : 2 * P])

                # ---- second half: Kt/V over slabs 2,3 ----
                kt_half(1, first_wk=wk1_first)
                v_half(1)

            # ---- attention: scores + exp + AV per 256-query block, in pairs
            #      so each Kt stationary load feeds two score matmuls ----
            with (
                tc.tile_pool(name="pt", bufs=3) as pt_pool,
                tc.tile_pool(name="ps", bufs=3, space="PSUM") as ps_pool,
                tc.tile_pool(name="po", bufs=4, space="PSUM") as po_pool,
                tc.tile_pool(name="pd", bufs=1, space="PSUM") as pd_pool,
                tc.tile_pool(name="ob", bufs=3) as ob_pool,
                tc.tile_pool(name="rc", bufs=4) as rc_pool,
            ):
                for a, b in ((0, 1), (2, 3)):
                    pts = {
                        q: pt_pool.tile([P, 16, QT], bf, tag="pt", name=f"pt{q}")
                        for q in (a, b)
                    }
                    for kc in range(4 * b + 4):
                        s, kq = kc // 4, kc % 4
                        active = [q for q in (a, b) if kc < 4 * q + 4]
                        pss = {
                            q: ps_pool.tile([P, QT], f32, tag="ps", name="ps")
                            for q in active
                        }
                        # columns j < 64*m of a band tile are masked for every
                        # partition -> skip them in the matmul entirely
                        j0s = {
                            q: max(0, 64 * (kc - 4 * q)) if kc >= 4 * q else 0
                            for q in active
                        }
                        for ec in range(8):
                            for q in active:
                                j0 = j0s[q]
                                nc.tensor.matmul(
                                    pss[q][:, j0:QT],
                                    kt_slabs[s][:, ec, kq * P : (kq + 1) * P],
                                    qt[:, ec, q * QT + j0 : (q + 1) * QT],
                                    start=(ec == 0),
                                    stop=(ec == 7),
                                )
                        for q in active:
                            m = kc - 4 * q
                            j0 = j0s[q]
                            if m >= 0:
                                nc.vector.tensor_add(
                                    out=pss[q][:, j0:QT],
                                    in0=pss[q][:, j0:QT],
                                    in1=deferred_loads["mask"][:, m, j0:QT],
                                )
                            nc.scalar.activation(
                                out=pts[q][:, kc, j0:QT],
                                in_=pss[q][:, j0:QT],
                                func=EXP,
                                scale=SCALE,
                            )
                            # zero the trimmed region that a later AV chunk reads
                            if m == 1:
                                nc.gpsimd.memset(pts[q][:, kc, 0:64], 0.0)
                            elif m == 3:
                                nc.gpsimd.memset(pts[q][:, kc, 128:192], 0.0)
                    for q in (b, a):
                        pt = pts[q]
                        for qc in range(2):
                            # P^T chunks 4q+2, 4q+3 are fully masked for the
                            # first 128-query half -> drop them from AV/denom
                            nk = 4 * q + 2 if qc == 0 else 4 * q + 4
                            dn = pd_pool.tile([P, 1], f32, tag="dn")
                            pos = [
                                po_pool.tile([P, 512], f32, tag="po", name="po")
                                for _ in range(2)
                            ]
                            for kc in range(nk):
                                s, kq = kc // 4, kc % 4
                                lhsT = pt[:, kc, qc * P : (qc + 1) * P]
                                first, last = kc == 0, kc == nk - 1
                                nc.tensor.matmul(
                                    dn[:], lhsT, deferred_loads["ones"][:],
                                    start=first, stop=last,
                                )
                                for es in range(2):
                                    nc.tensor.matmul(
                                        pos[es][:],
                                        lhsT,
                                        v_slabs[s][:, kq, es * 512 : (es + 1) * 512],
                                        start=first,
                                        stop=last,
                                    )
                            rc = rc_pool.tile([P, 1], f32, tag="rc")
                            nc.vector.reciprocal(out=rc[:], in_=dn[:])
                            q0 = q * QT + qc * P
                            for es in range(2):
                                ob = ob_pool.tile([P, 512], f32, tag="ob")
                                if es == 0:
                                    nc.vector.tensor_scalar_mul(
                                        out=ob[:], in0=pos[es][:], scalar1=rc[:]
                                    )
                                else:
                                    nc.scalar.activation(
                                        out=ob[:], in_=pos[es][:],
                                        func=COPYF, scale=rc[:],
                                    )
                                nc.sync.dma_start(
                                    out_ap[q0 : q0 + P, es * 512 : (es + 1) * 512],
                                    ob[:],
                                )

    nc.compile()
    return nc


def _get_nc():
    if "nc" not in _cache:
        _cache["nc"] = _build()
    return _cache["nc"]


def _host_masks(h: int) -> np.ndarray:
    # mask[m, p, j]: score tile at key chunk kc = 4*i+m, query block i.
    # global k = 512*i + 128*m + p, global q = 2*(256*i + j) + h.
    # keep (0.0) iff k <= q  <=>  p - 2j <= h - 128*m, else -1e30.
    m = np.arange(4)[:, None, None]
    p = np.arange(P)[None, :, None]
    j = np.arange(QT)[None, None, :]
    keep = (p - 2 * j) <= (h - 128 * m)
    return np.where(keep, np.float32(0.0), np.float32(NEG)).astype(np.float32)


def make_in_maps(x, Wq, Wk, Wv):
    import ml_dtypes

    bf = ml_dtypes.bfloat16
    wqT = np.ascontiguousarray(Wq.T).astype(bf)
    wkT = np.ascontiguousarray(Wk.T).astype(bf)
    wvT = np.ascontiguousarray(Wv.T).astype(bf)
    masks_h = [_host_masks(0), _host_masks(1)]
    in_maps = []
    for c in range(N_CORES):
        b, h = c // 2, c % 2
        xb = np.asarray(x[b], dtype=np.float32)
        in_maps.append(
            {
                "xT": np.ascontiguousarray(xb.T).astype(bf),
                "xqT": np.ascontiguousarray(xb[h::2, :].T).astype(bf),
                "wqT": wqT,
                "wkT": wkT,
                "wvT": wvT,
                "masks": masks_h[h],
            }
        )
    return in_maps


def kernel(x, Wq, Wk, Wv):
    from concourse.bass_utils import run_bass_kernel_spmd

    nc = _get_nc()
    in_maps = make_in_maps(x, Wq, Wk, Wv)
    res = run_bass_kernel_spmd(nc, in_maps, core_ids=list(range(N_CORES)))
    out = np.empty((B, S, D), dtype=np.float32)
    for c in range(N_CORES):
        b, h = c // 2, c % 2
        out[b, h::2, :] = res.results[c]["out"]
    return out
